# revision 10
# baseline (speedup 1.0000x reference)
"""Trainium2 Bass kernel for nn_BDH_GPU (sparse linear-attention decoder).

Self-contained: builds an SPMD Bass/Tile program for 8 NeuronCores,
shards batch(2) x head-groups(4), runs via PJRT (axon), gathers output.

Sharding: core c -> batch b=c//4, heads [4*(c%4), 4*(c%4)+4).
Per-layer AllReduce of y@encoder partials within each 4-core group.

v2 design vs v1 (baseline):
- encoder SBUF-resident (it is layer-invariant), as 16 natural [128, D]
  tiles; y@encoder emits w in NATURAL [t, d] layout so all LayerNorm
  reductions run along the free axis (DVE tensor_reduce), with a
  closed-form outer norm: msq(s) = 2 + (2/D) rsw <v,w> - eps rsw^2
- bf16 master v (no fp32 copy); vnew = v*rss + w*(rsw*rss) - mw*rsw*rss
- T-half layer pipeline: the first T-half of layer l+1 only depends on
  the first T-half of layer l (causal attention), so each of the two
  per-layer AllReduce halves overlaps a ~100us window of independent
  PE work; chunked RoPE; jj-outer x/y emission
"""
import numpy as np
import ml_dtypes

import concourse.bass as bass
import concourse.tile as tile
import concourse.mybir as mybir
from concourse import bacc, bass2jax

AF = mybir.ActivationFunctionType
ALU = mybir.AluOpType
AX = mybir.AxisListType
FP32 = mybir.dt.float32
BF16 = mybir.dt.bfloat16
ts = bass.ts

D, H, N, VOCAB, L, SD, B, T = 1024, 16, 8192, 32000, 4, 512, 2, 1024
NCORES = 8
NHC = 4           # heads per core
VSH = VOCAB // 4  # vocab shard per core (within batch group) = 8000
VCH = 500         # vocab N-chunk (<=512 f32 psum bank)
NVC = VSH // VCH  # 16
EPS = 1e-5
T2 = T // 2

_CACHE = {}


def build_program(nlayers=L, repeat=1, do_readout=True, collective=True):
    nc = bacc.Bacc("TRN2", target_bir_lowering=False, debug=False,
                   num_devices=NCORES)
    CDT = BF16

    v0t_c = nc.dram_tensor("v0t_c", [D, T], CDT, kind="ExternalInput")
    v0n_c = nc.dram_tensor("v0n_c", [T, D], CDT, kind="ExternalInput")
    wx_d = nc.dram_tensor("wx", [NHC, D, SD], CDT, kind="ExternalInput")
    wy_d = nc.dram_tensor("wy", [NHC, D, SD], CDT, kind="ExternalInput")
    enc_d = nc.dram_tensor("enc", [NHC * SD, D], CDT, kind="ExternalInput")
    ro_d = nc.dram_tensor("ro", [D, VSH], CDT, kind="ExternalInput")
    cos_d = nc.dram_tensor("cos", [SD // 2, T], CDT, kind="ExternalInput")
    sin_d = nc.dram_tensor("sin", [SD // 2, T], CDT, kind="ExternalInput")
    msk_d = nc.dram_tensor("msk", [2, 128, 256], CDT, kind="ExternalInput")
    out_d = nc.dram_tensor("logits", [T, VSH], FP32, kind="ExternalOutput")

    with tile.TileContext(nc) as tc:
        with (
            tc.tile_pool(name="res", bufs=1) as res,
            tc.tile_pool(name="act", bufs=1) as act,
            tc.tile_pool(name="wst", bufs=10) as wst,
            tc.tile_pool(name="sml", bufs=2) as sml,
            tc.tile_pool(name="tlp", bufs=2) as tlp,
            tc.tile_pool(name="stg", bufs=4) as stg,
            tc.tile_pool(name="psp", bufs=2, space="PSUM") as psp,
            tc.tile_pool(name="dram", bufs=2, space="DRAM") as dram,
        ):
            def P5(nm):  # x/y/readout psum [128,512]
                return psp.tile([128, 512], FP32, tag="p5", bufs=2, name=nm)

            def PE5(nm):  # enc psum [128,512]
                return psp.tile([128, 512], FP32, tag="ep", bufs=2, name=nm)

            def PSC(nm):  # score psum [128,256]
                return psp.tile([128, 256], FP32, tag="scp", bufs=2, name=nm)

            def PA(nm):  # a=V@s psum [128,256]
                return psp.tile([128, 256], FP32, tag="ap", bufs=2, name=nm)

            def PST(nm):  # stp psum [128,256] (shares the score ring)
                return psp.tile([128, 256], FP32, tag="scp", bufs=2, name=nm)

            # ---- constants ----
            cosv, sinv, masks = [], [], []
            for i in range(2):
                ct = res.tile([128, T], CDT, name=f"cos{i}")
                nc.sync.dma_start(ct[:], cos_d[ts(i, 128), :])
                cosv.append(ct)
                st = res.tile([128, T], CDT, name=f"sin{i}")
                nc.sync.dma_start(st[:], sin_d[ts(i, 128), :])
                sinv.append(st)
            for i in range(2):
                mt = res.tile([128, 256], CDT, name=f"msk{i}")
                nc.sync.dma_start(mt[:], msk_d[i])
                masks.append(mt)
            ones = res.tile([128, 128], CDT, name="ones")
            nc.vector.memset(ones[:], 1.0)
            epst = res.tile([128, 1], FP32, name="epst")
            nc.vector.memset(epst[:], EPS)

            # ---- resident encoder: 16 x [128, D] (natural rows = n) ----
            enc_t = []
            for kk in range(16):
                e = res.tile([128, D], CDT, name=f"enc{kk}")
                nc.sync.dma_start(e[:], enc_d[ts(kk, 128), :])
                enc_t.append(e)

            # ---- persistent v tiles (updated in place each layer) ----
            vT_c = [res.tile([128, T], CDT, name=f"vTc{k}") for k in range(8)]
            vn_c = [res.tile([128, D], CDT, name=f"vnc{k}") for k in range(8)]

            def load_v(sfx):
                for k in range(8):
                    nc.sync.dma_start(vT_c[k][:], v0t_c[ts(k, 128), :])
                for k in range(8):
                    nc.sync.dma_start(vn_c[k][:], v0n_c[ts(k, 128), :])

            def tiny(nm):
                return tlp.tile([128, 1], FP32, tag="tiny", bufs=32, name=nm)

            # ---------------- emission helpers ----------------
            def emit_x_mm(h, jj, xy, tg):
                """x matmuls + relu for head h, T-half jj (fresh wx stream)."""
                wxt = []
                for k in range(8):
                    w = wst.tile([128, SD], CDT, tag="wxs", bufs=8,
                                 name=f"wx_{tg}h{h}j{jj}k{k}")
                    nc.sync.dma_start(w[:], wx_d[h, ts(k, 128), :])
                    wxt.append(w)
                for m in range(4):
                    ps = P5(f"xps_{tg}h{h}m{m}j{jj}")
                    for k in range(8):
                        nc.tensor.matmul(ps[:], wxt[k][:, ts(m, 128)],
                                         vT_c[k][:, ts(jj, 512)],
                                         start=(k == 0), stop=(k == 7))
                    nc.scalar.activation(out=xy[m][:, ts(jj, 512)], in_=ps[:],
                                         func=AF.Relu)

            def emit_rope(h, jj, xy, qrh, tg):
                """rope for head h half jj -> qrh[i][:, 0:512]."""
                sl = ts(jj, 512)
                for i in range(2):
                    t1 = sml.tile([128, 512], CDT, tag="ropet", bufs=2,
                                  name=f"t1_{tg}h{h}j{jj}i{i}")
                    nc.gpsimd.tensor_mul(t1[:], xy[i][:, sl], cosv[i][:, sl])
                    nc.gpsimd.tensor_mul(qrh[i][:], xy[2 + i][:, sl],
                                         sinv[i][:, sl])
                    nc.vector.tensor_sub(qrh[i][:], t1[:], qrh[i][:])
                    t3 = sml.tile([128, 512], CDT, tag="ropet", bufs=2,
                                  name=f"t3_{tg}h{h}j{jj}i{i}")
                    nc.gpsimd.tensor_mul(t3[:], xy[i][:, sl], sinv[i][:, sl])
                    nc.gpsimd.tensor_mul(qrh[2 + i][:], xy[2 + i][:, sl],
                                         cosv[i][:, sl])
                    nc.vector.tensor_add(qrh[2 + i][:], t3[:], qrh[2 + i][:])

            def emit_attn(h, j, qrA, qrB, alnh, tg):
                """scores(j) -> a(j) -> stp(j) -> rs -> alnh[:, (j%2)*256]."""
                nsb = 2 * j + 2
                rhs_q = qrA if j < 2 else qrB
                rj = ts(j % 2, 256)

                def qsl(kq, i):  # lhsT slice for s-tile i
                    if i < 4:
                        return qrA[kq][:, ts(i, 128)]
                    return qrB[kq][:, ts(i - 4, 128)]

                sc = []
                for i in range(nsb):
                    s = sml.tile([128, 256], CDT, tag=f"sc{i}", bufs=2,
                                 name=f"sc{i}_{tg}h{h}j{j}")
                    sc.append(s)
                    ps = PSC(f"scp_{tg}h{h}j{j}i{i}")
                    for kq in range(4):
                        nc.tensor.matmul(ps[:], qsl(kq, i), rhs_q[kq][:, rj],
                                         start=(kq == 0), stop=(kq == 3))
                    if i >= 2 * j:
                        nc.vector.tensor_mul(s[:], ps[:], masks[i - 2 * j][:])
                    else:
                        nc.scalar.activation(out=s[:], in_=ps[:], func=AF.Copy)
                afs, sqs = [], []
                for d8 in range(8):
                    ap = PA(f"ap_{tg}h{h}j{j}d{d8}")
                    for i in range(nsb):
                        nc.tensor.matmul(ap[:], vn_c[i][:, ts(d8, 128)],
                                         sc[i][:],
                                         start=(i == 0), stop=(i == nsb - 1))
                    af = sml.tile([128, 256], CDT, tag=f"af{d8}", bufs=1,
                                  name=f"af_{tg}h{h}j{j}d{d8}")
                    sq = sml.tile([128, 256], CDT, tag="sq", bufs=8,
                                  name=f"sq_{tg}h{h}j{j}d{d8}")
                    if d8 % 2 == 0:
                        nc.scalar.activation(out=af[:], in_=ap[:], func=AF.Copy)
                        nc.gpsimd.tensor_mul(sq[:], af[:], af[:])
                    else:
                        nc.vector.tensor_copy(af[:], ap[:])
                        nc.scalar.activation(out=sq[:], in_=ap[:],
                                             func=AF.Square)
                    afs.append(af)
                    sqs.append(sq)
                stp = PST(f"stp_{tg}h{h}j{j}")
                for d8 in range(8):
                    nc.tensor.matmul(stp[:], ones[:], sqs[d8][:],
                                     start=(d8 == 0), stop=(d8 == 7))
                rs = sml.tile([128, 256], FP32, tag="rs", bufs=2,
                              name=f"rs_{tg}h{h}j{j}")
                nc.scalar.activation(out=rs[:], in_=stp[:], func=AF.Sqrt,
                                     bias=epst[:], scale=1.0 / D)
                nc.vector.reciprocal(rs[:], rs[:])
                for d8 in range(8):
                    eng = nc.vector if d8 % 2 == 0 else nc.gpsimd
                    eng.tensor_mul(alnh[d8][:, rj], afs[d8][:], rs[:])

            def emit_y(h, jj, xy, alnh, tg):
                """y = relu(Wy^T @ aln_half) * x into xy in place."""
                wyt = []
                for k in range(8):
                    w = wst.tile([128, SD], CDT, tag="wys", bufs=8,
                                 name=f"wy_{tg}h{h}j{jj}k{k}")
                    nc.sync.dma_start(w[:], wy_d[h, ts(k, 128), :])
                    wyt.append(w)
                for m in range(4):
                    ps = P5(f"yps_{tg}h{h}m{m}j{jj}")
                    for k in range(8):
                        nc.tensor.matmul(ps[:], wyt[k][:, ts(m, 128)],
                                         alnh[k][:],
                                         start=(k == 0), stop=(k == 7))
                    rl = sml.tile([128, 512], CDT, tag="rl", bufs=2,
                                  name=f"rl_{tg}h{h}m{m}j{jj}")
                    nc.scalar.activation(out=rl[:], in_=ps[:], func=AF.Relu)
                    nc.vector.tensor_mul(xy[m][:, ts(jj, 512)], rl[:],
                                         xy[m][:, ts(jj, 512)])

            def emit_enc(ti_range, xys, ar_in, tg):
                """w[t,d] partials: lhsT = y t-slices, rhs = resident enc."""
                for ti in ti_range:
                    for dh in range(2):
                        ps = PE5(f"ep_{tg}t{ti}d{dh}")
                        for kk in range(16):
                            h, m = kk // 4, kk % 4
                            nc.tensor.matmul(
                                ps[:], xys[h][m][:, ts(ti, 128)],
                                enc_t[kk][:, ts(dh, 512)],
                                start=(kk == 0), stop=(kk == 15))
                        so = stg.tile([128, 512], CDT, tag="so", bufs=2,
                                      name=f"so_{tg}t{ti}d{dh}")
                        if dh == 0:
                            nc.scalar.activation(out=so[:], in_=ps[:],
                                                 func=AF.Copy)
                        else:
                            nc.vector.tensor_copy(so[:], ps[:])
                        nc.sync.dma_start(
                            ar_in[ts(ti % 4, 128), ts(dh, 512)], so[:])

            def emit_tail(ti, ar_out, tg):
                """w chunk -> free-axis LN stats -> vnew in place."""
                wb = tlp.tile([128, D], CDT, tag="wb", bufs=3,
                              name=f"wb_{tg}t{ti}")
                nc.sync.dma_start(wb[:], ar_out[ts(ti % 4, 128), :])
                mw_r = tiny(f"mwr_{tg}t{ti}")
                nc.vector.tensor_reduce(mw_r[:], wb[:], axis=AX.X, op=ALU.add)
                tsc = tlp.tile([128, D], CDT, tag="tts", bufs=1,
                               name=f"tsc_{tg}t{ti}")
                ms_r = tiny(f"msr_{tg}t{ti}")
                nc.scalar.activation(out=tsc[:], in_=wb[:], func=AF.Square,
                                     accum_out=ms_r[:])
                tsv = tlp.tile([128, D], CDT, tag="tts", bufs=1,
                               name=f"tsv_{tg}t{ti}")
                vw_r = tiny(f"vwr_{tg}t{ti}")
                nc.gpsimd.tensor_mul(tsv[:], wb[:], vn_c[ti][:])
                nc.vector.tensor_reduce(vw_r[:], tsv[:], axis=AX.X,
                                        op=ALU.add)
                # scalar chain on [128,1]
                mw = tiny(f"mw_{tg}t{ti}")
                nc.scalar.activation(out=mw[:], in_=mw_r[:], func=AF.Copy,
                                     scale=1.0 / D)
                m2 = tiny(f"m2_{tg}t{ti}")
                nc.scalar.activation(out=m2[:], in_=mw_r[:], func=AF.Square,
                                     scale=1.0 / D)
                varw = tiny(f"varw_{tg}t{ti}")
                nc.scalar.activation(out=varw[:], in_=ms_r[:], func=AF.Copy,
                                     scale=1.0 / D)
                nc.vector.tensor_sub(varw[:], varw[:], m2[:])
                rsw = tiny(f"rsw_{tg}t{ti}")
                nc.scalar.activation(out=rsw[:], in_=varw[:], func=AF.Sqrt,
                                     bias=epst[:], scale=1.0)
                nc.vector.reciprocal(rsw[:], rsw[:])
                r2 = tiny(f"r2_{tg}t{ti}")
                nc.vector.tensor_mul(r2[:], rsw[:], rsw[:])
                g = tiny(f"g_{tg}t{ti}")
                nc.scalar.activation(out=g[:], in_=r2[:], func=AF.Copy,
                                     scale=-EPS, bias=2.0)
                p = tiny(f"p_{tg}t{ti}")
                nc.scalar.activation(out=p[:], in_=vw_r[:], func=AF.Copy,
                                     scale=2.0 / D)
                nc.vector.tensor_mul(p[:], p[:], rsw[:])
                msq = tiny(f"msq_{tg}t{ti}")
                nc.vector.tensor_add(msq[:], p[:], g[:])
                rss = tiny(f"rss_{tg}t{ti}")
                nc.scalar.activation(out=rss[:], in_=msq[:], func=AF.Sqrt,
                                     bias=epst[:], scale=1.0)
                nc.vector.reciprocal(rss[:], rss[:])
                bsc = tiny(f"bsc_{tg}t{ti}")
                nc.vector.tensor_mul(bsc[:], rsw[:], rss[:])
                cng = tiny(f"cng_{tg}t{ti}")
                nc.vector.tensor_mul(cng[:], mw[:], bsc[:])
                nc.scalar.activation(out=cng[:], in_=cng[:], func=AF.Copy,
                                     scale=-1.0)
                # vnew = v*rss + w*bsc + cng  (in place into vn_c[ti])
                t1 = tlp.tile([128, D], CDT, tag="t1", bufs=2,
                              name=f"t1_{tg}t{ti}")
                nc.vector.tensor_scalar(t1[:], wb[:], bsc[:], cng[:],
                                        ALU.mult, ALU.add)
                t2 = tlp.tile([128, D], CDT, tag="t2", bufs=2,
                              name=f"t2_{tg}t{ti}")
                nc.scalar.activation(out=t2[:], in_=vn_c[ti][:], func=AF.Copy,
                                     scale=rss[:])
                nc.gpsimd.tensor_add(vn_c[ti][:], t1[:], t2[:])
                for a in range(8):
                    nc.sync.dma_start_transpose(
                        vT_c[a][:, ts(ti, 128)], vn_c[ti][:, ts(a, 128)])

            # ---------------- program ----------------
            st = {}

            def QA(h, tg):
                return [act.tile([128, 512], CDT, tag=f"qA{i}", bufs=4,
                                 name=f"qA{i}_{tg}h{h}") for i in range(4)]

            def QB(h, tg):
                return [act.tile([128, 512], CDT, tag=f"qB{i}", bufs=2,
                                 name=f"qB{i}_{tg}h{h}") for i in range(4)]

            def pre_emit_x(tg):
                """x(jj0) for all heads + rope jj0 for all heads."""
                xys, qAs = [], []
                for h in range(4):
                    xy = [act.tile([128, T], CDT, tag=f"xy{h}_{m}", bufs=1,
                                   name=f"xy{h}_{m}_{tg}") for m in range(4)]
                    xys.append(xy)
                    emit_x_mm(h, 0, xy, tg)
                    qa = QA(h, tg)
                    emit_rope(h, 0, xy, qa, tg)
                    qAs.append(qa)
                st["xys"] = xys
                st["qAs"] = qAs

            def cc(ar_in, ar_out):
                nc.gpsimd.collective_compute(
                    "AllReduce", ALU.add,
                    replica_groups=[[0, 1, 2, 3], [4, 5, 6, 7]],
                    ins=[ar_in.opt()], outs=[ar_out.opt()])

            load_v("init")
            pre_emit_x("r0pre")
            pending_tail = None  # (ar_out_half2, tag) from previous layer

            for rep in range(repeat):
                for layer in range(nlayers):
                    tg = f"r{rep}l{layer}"
                    xys, qAs = st["xys"], st["qAs"]
                    ADT = CDT
                    ar_in = [dram.tile([T2, D], ADT, tag=f"ar_in{q}",
                                       name=f"ari{q}_{tg}") for q in range(2)]
                    ar_out = [dram.tile([T2, D], ADT, tag=f"ar_out{q}",
                                        name=f"aro{q}_{tg}") for q in range(2)]
                    w0 = ar_out[0] if collective else ar_in[0]
                    w1 = ar_out[1] if collective else ar_in[1]

                    # ---- half 1: attn j0/j1, y jj0, enc t0..3, CC1 ----
                    alnAs = []
                    for h in range(4):
                        alnh = [act.tile([128, 512], CDT, tag=f"alA{k}",
                                         bufs=1, name=f"alA{k}_{tg}h{h}")
                                for k in range(8)]
                        alnAs.append(alnh)
                        emit_attn(h, 0, qAs[h], None, alnh, tg)
                        emit_attn(h, 1, qAs[h], None, alnh, tg)
                        emit_y(h, 0, xys[h], alnh, tg)
                        if h == 3 and pending_tail is not None:
                            par, ptg = pending_tail
                            for ti in range(4, 8):
                                emit_tail(ti, par, ptg)
                            pending_tail = None
                    if pending_tail is not None:  # layer 0 of a rep
                        par, ptg = pending_tail
                        for ti in range(4, 8):
                            emit_tail(ti, par, ptg)
                        pending_tail = None
                    emit_enc(range(0, 4), xys, ar_in[0], tg)
                    if collective:
                        cc(ar_in[0], ar_out[0])

                    # ---- half 2: x jj1, attn j2/j3, y jj1 ----
                    qB_prev = None
                    for h in range(4):
                        if h == 0:
                            qb = QB(0, tg)
                            emit_x_mm(0, 1, xys[0], tg)
                            emit_rope(0, 1, xys[0], qb, tg)
                        else:
                            qb = qB_prev
                        if h < 3:
                            qb_next = QB(h + 1, tg)
                            emit_x_mm(h + 1, 1, xys[h + 1], tg)
                            emit_rope(h + 1, 1, xys[h + 1], qb_next, tg)
                            qB_prev = qb_next
                        alnh = [act.tile([128, 512], CDT, tag=f"alB{k}",
                                         bufs=1, name=f"alB{k}_{tg}h{h}")
                                for k in range(8)]
                        emit_attn(h, 2, qAs[h], qb, alnh, tg)
                        emit_attn(h, 3, qAs[h], qb, alnh, tg)
                        emit_y(h, 1, xys[h], alnh, tg)

                    # ---- tail half1 (waits CC1; runs during enc t4..7) ----
                    for ti in range(0, 4):
                        emit_tail(ti, w0, tg)
                    emit_enc(range(4, 8), xys, ar_in[1], tg)
                    if collective:
                        cc(ar_in[1], ar_out[1])

                    last_layer = (layer == nlayers - 1)
                    last = last_layer and (rep == repeat - 1)
                    if not last_layer:
                        pre_emit_x(f"{tg}pre")
                        pending_tail = (w1, tg)
                    elif not last:
                        # rep boundary: drain tail, reload v, restart
                        for ti in range(4, 8):
                            emit_tail(ti, w1, tg)
                        load_v(f"r{rep+1}")
                        pre_emit_x(f"r{rep+1}pre")
                    else:
                        for ti in range(4, 8):
                            emit_tail(ti, w1, tg)

            # ---- readout: logits = v^T @ readout_shard ----
            if do_readout:
                for nn_ in range(NVC):
                    rot = []
                    for k in range(8):
                        rtile = wst.tile([128, 512], CDT, tag="wys", bufs=8,
                                         name=f"ro_n{nn_}k{k}")
                        nc.sync.dma_start(
                            rtile[:, 0:VCH], ro_d[ts(k, 128), ts(nn_, VCH)])
                        rot.append(rtile)
                    for m in range(8):
                        ps = P5(f"rps_n{nn_}m{m}")
                        for k in range(8):
                            nc.tensor.matmul(ps[:, 0:VCH],
                                             vT_c[k][:, ts(m, 128)],
                                             rot[k][:, 0:VCH],
                                             start=(k == 0), stop=(k == 7))
                        ot = stg.tile([128, VCH], FP32, tag="ot", bufs=2,
                                      name=f"ot_n{nn_}m{m}")
                        if m % 2 == 0:
                            nc.vector.tensor_copy(ot[:], ps[:, 0:VCH])
                        else:
                            nc.scalar.activation(out=ot[:], in_=ps[:, 0:VCH],
                                                 func=AF.Copy)
                        nc.sync.dma_start(
                            out_d[ts(m, 128), ts(nn_, VCH)], ot[:])
    nc.compile()
    return nc


def host_prep(inputs):
    idx = np.asarray(inputs["idx"])
    wte = np.asarray(inputs["wte"], np.float32)
    enc = np.asarray(inputs["encoder"], np.float32)
    dx = np.asarray(inputs["decoder_x"], np.float32)
    dy = np.asarray(inputs["decoder_y"], np.float32)
    ro = np.asarray(inputs["readout"], np.float32)
    bf = ml_dtypes.bfloat16

    perm = np.concatenate([np.arange(0, SD, 2), np.arange(1, SD, 2)])
    Wx = np.ascontiguousarray(dx[:, :, perm])                       # [H, D, SD]
    Wy = np.ascontiguousarray(dy[:, :, perm])
    encp = np.ascontiguousarray(enc.reshape(H, SD, D)[:, perm, :])  # [H, SD, D]

    g = wte[idx]                                                    # [B, T, D]
    m = g.mean(-1, keepdims=True)
    var = ((g - m) ** 2).mean(-1, keepdims=True)
    v0 = (g - m) / np.sqrt(var + EPS)

    inv_freq = 1.0 / (10000.0 ** (np.arange(0, SD, 2, dtype=np.float32) / SD))
    freqs = np.arange(T, dtype=np.float32)[None, :] * inv_freq[:, None]
    cosT = np.cos(freqs).astype(np.float32)                         # [SD/2, T]
    sinT = np.sin(freqs).astype(np.float32)

    ss, tt = np.mgrid[0:128, 0:256]
    msk = np.stack([(tt > ss), (tt > ss + 128)]).astype(np.float32)

    in_maps = []
    for c in range(NCORES):
        b, hs = c // 4, c % 4
        hsl = slice(4 * hs, 4 * hs + 4)
        v0T = np.ascontiguousarray(v0[b].T)
        in_maps.append({
            "v0t_c": v0T.astype(bf),
            "v0n_c": np.ascontiguousarray(v0[b]).astype(bf),
            "wx": Wx[hsl].astype(bf),
            "wy": Wy[hsl].astype(bf),
            "enc": np.ascontiguousarray(encp[hsl].reshape(NHC * SD, D)).astype(bf),
            "ro": np.ascontiguousarray(ro[:, VSH * hs: VSH * (hs + 1)]).astype(bf),
            "cos": cosT.astype(bf),
            "sin": sinT.astype(bf),
            "msk": msk.astype(bf),
        })
    return in_maps


def make_runner(nc, n_cores=NCORES):
    import jax
    from jax.sharding import Mesh, PartitionSpec
    from jax.experimental.shard_map import shard_map

    bass2jax.install_neuronx_cc_hook()
    partition_name = nc.partition_id_tensor.name if nc.partition_id_tensor else None
    in_names, out_names, out_avals, zero_shapes = [], [], [], []
    for alloc in nc.m.functions[0].allocations:
        if not isinstance(alloc, mybir.MemoryLocationSet):
            continue
        name = alloc.memorylocations[0].name
        if alloc.kind == "ExternalInput":
            if name != partition_name:
                in_names.append(name)
        elif alloc.kind == "ExternalOutput":
            shape = tuple(alloc.tensor_shape)
            dtype = mybir.dt.np(alloc.dtype)
            out_names.append(name)
            out_avals.append(jax.core.ShapedArray(shape, dtype))
            zero_shapes.append((shape, dtype))
    n_params, n_outs = len(in_names), len(out_avals)
    all_in = list(in_names) + list(out_names)
    if partition_name is not None:
        all_in.append(partition_name)

    def _body(*args):
        operands = list(args)
        if partition_name is not None:
            operands.append(bass2jax.partition_id_tensor())
        return tuple(bass2jax._bass_exec_p.bind(
            *operands, out_avals=tuple(out_avals), in_names=tuple(all_in),
            out_names=tuple(out_names), lowering_input_output_aliases=(),
            sim_require_finite=True, sim_require_nnan=True, nc=nc))

    devices = jax.devices()[:n_cores]
    mesh = Mesh(np.asarray(devices), ("core",))
    f = jax.jit(
        shard_map(_body, mesh=mesh,
                  in_specs=(PartitionSpec("core"),) * (n_params + n_outs),
                  out_specs=(PartitionSpec("core"),) * n_outs, check_rep=False),
        keep_unused=True)

    def prep(in_maps):
        concat = [np.concatenate([np.asarray(in_maps[c][k])
                                  for c in range(n_cores)], axis=0)
                  for k in in_names]
        zeros = [np.zeros((n_cores * s[0], *s[1:]), dt) for (s, dt) in zero_shapes]
        return [jax.device_put(x) for x in concat + zeros]

    def run(dev_args):
        outs = f(*dev_args)
        jax.block_until_ready(outs)
        return outs

    def split(outs):
        return [{name: np.asarray(outs[i]).reshape(n_cores, *out_avals[i].shape)[c]
                 for i, name in enumerate(out_names)} for c in range(n_cores)]

    return run, prep, split


def kernel(**inputs) -> np.ndarray:
    if "prog" not in _CACHE:
        nc = build_program()
        _CACHE["prog"] = nc
        _CACHE["runner"] = make_runner(nc)
    run, prep, split = _CACHE["runner"]
    in_maps = host_prep(inputs)
    args = prep(in_maps)
    res = split(run(args))
    out = np.zeros((B, T, VOCAB), np.float32)
    for c in range(NCORES):
        b, hs = c // 4, c % 4
        out[b, :, VSH * hs: VSH * (hs + 1)] = res[c]["logits"]
    return out


# revision 16
# speedup vs baseline: 1.4695x; 1.4695x over previous
"""Trainium2 Bass kernel for nn_BDH_GPU (sparse linear-attention decoder).

Self-contained: builds an SPMD Bass/Tile program for 8 NeuronCores,
shards batch(2) x head-groups(4), runs via PJRT (axon), gathers output.

Sharding: core c -> batch b=c//4, heads [4*(c%4), 4*(c%4)+4).
Per-layer AllReduce of y@encoder partials within each 4-core group.

v4 design vs v1 (baseline):
- encoder SBUF-resident (it is layer-invariant), as 16 natural [128, D]
  tiles; y@encoder emits w in NATURAL [t, d] layout so all LayerNorm
  reductions run along the free axis, with a closed-form outer norm:
  msq(s) = 2 + (2/D) rsw <v,w> - eps rsw^2
- bf16 master v (no fp32 copy); vnew = v*rss + w*(rsw*rss) - mw*rsw*rss
- LN(a) scale folded past the relu: y = (relu(Wy^T a) * rs) * x since
  relu(z*rs) = rs*relu(z) for rs > 0 — removes all aln-normalize muls
- T-half layer pipeline: the first T-half of layer l+1 only depends on
  the first T-half of layer l (causal attention), so each of the two
  per-layer AllReduce halves overlaps ~100us of independent PE work
"""
import numpy as np
import ml_dtypes

import concourse.bass as bass
import concourse.tile as tile
import concourse.mybir as mybir
from concourse import bacc, bass2jax

AF = mybir.ActivationFunctionType
ALU = mybir.AluOpType
AX = mybir.AxisListType
FP32 = mybir.dt.float32
BF16 = mybir.dt.bfloat16
ts = bass.ts

D, H, N, VOCAB, L, SD, B, T = 1024, 16, 8192, 32000, 4, 512, 2, 1024
NCORES = 8
NHC = 4
VSH = VOCAB // 4
VCH = 500
NVC = VSH // VCH
EPS = 1e-5
T2 = T // 2

_CACHE = {}


def build_program(nlayers=L, repeat=1, do_readout=True, collective=True):
    nc = bacc.Bacc("TRN2", target_bir_lowering=False, debug=False,
                   num_devices=NCORES)
    CDT = BF16

    v0t_c = nc.dram_tensor("v0t_c", [D, T], CDT, kind="ExternalInput")
    v0n_c = nc.dram_tensor("v0n_c", [T, D], CDT, kind="ExternalInput")
    wx_d = nc.dram_tensor("wx", [NHC, D, SD], CDT, kind="ExternalInput")
    wy_d = nc.dram_tensor("wy", [NHC, D, SD], CDT, kind="ExternalInput")
    enc_d = nc.dram_tensor("enc", [NHC * SD, D], CDT, kind="ExternalInput")
    ro_d = nc.dram_tensor("ro", [D, VSH], CDT, kind="ExternalInput")
    cos_d = nc.dram_tensor("cos", [SD // 2, T], CDT, kind="ExternalInput")
    sin_d = nc.dram_tensor("sin", [SD // 2, T], CDT, kind="ExternalInput")
    msk_d = nc.dram_tensor("msk", [2, 128, 256], CDT, kind="ExternalInput")
    out_d = nc.dram_tensor("logits", [T, VSH], FP32, kind="ExternalOutput")

    with tile.TileContext(nc) as tc:
        with (
            tc.tile_pool(name="res", bufs=1) as res,
            tc.tile_pool(name="act", bufs=1) as act,
            tc.tile_pool(name="wst", bufs=10) as wst,
            tc.tile_pool(name="sml", bufs=2) as sml,
            tc.tile_pool(name="tlp", bufs=2) as tlp,
            tc.tile_pool(name="stg", bufs=4) as stg,
            tc.tile_pool(name="psp", bufs=2, space="PSUM") as psp,
            tc.tile_pool(name="dram", bufs=2, space="DRAM") as dram,
        ):
            def P5(nm):
                return psp.tile([128, 512], FP32, tag="p5", bufs=2, name=nm)

            def PE5(nm):
                return psp.tile([128, 512], FP32, tag="ep", bufs=2, name=nm)

            def PSC(nm):
                return psp.tile([128, 256], FP32, tag="scp", bufs=2, name=nm)

            def PA(nm):
                return psp.tile([128, 256], FP32, tag="ap", bufs=2, name=nm)

            def PST(nm):
                return psp.tile([128, 256], FP32, tag="scp", bufs=2, name=nm)

            cosv, sinv, masks = [], [], []
            for i in range(2):
                ct = res.tile([128, T], CDT, name=f"cos{i}")
                nc.sync.dma_start(ct[:], cos_d[ts(i, 128), :])
                cosv.append(ct)
                st = res.tile([128, T], CDT, name=f"sin{i}")
                nc.sync.dma_start(st[:], sin_d[ts(i, 128), :])
                sinv.append(st)
            for i in range(2):
                mt = res.tile([128, 256], CDT, name=f"msk{i}")
                nc.sync.dma_start(mt[:], msk_d[i])
                masks.append(mt)
            ones = res.tile([128, 128], CDT, name="ones")
            nc.vector.memset(ones[:], 1.0)
            epst = res.tile([128, 1], FP32, name="epst")
            nc.vector.memset(epst[:], EPS)

            enc_t = []
            for kk in range(16):
                e = res.tile([128, D], CDT, name=f"enc{kk}")
                nc.sync.dma_start(e[:], enc_d[ts(kk, 128), :])
                enc_t.append(e)

            vT_c = [res.tile([128, T], CDT, name=f"vTc{k}") for k in range(8)]
            vn_c = [res.tile([128, D], CDT, name=f"vnc{k}") for k in range(8)]

            def load_v(sfx):
                for k in range(8):
                    nc.sync.dma_start(vT_c[k][:], v0t_c[ts(k, 128), :])
                for k in range(8):
                    nc.sync.dma_start(vn_c[k][:], v0n_c[ts(k, 128), :])

            def tiny(nm):
                return tlp.tile([128, 1], FP32, tag="tiny", bufs=32, name=nm)

            def emit_x_mm(h, jj, xy, tg):
                wxt = []
                for k in range(8):
                    w = wst.tile([128, SD], CDT, tag="wxs", bufs=8,
                                 name=f"wx_{tg}h{h}j{jj}k{k}")
                    nc.sync.dma_start(w[:], wx_d[h, ts(k, 128), :])
                    wxt.append(w)
                for m in range(4):
                    ps = P5(f"xps_{tg}h{h}m{m}j{jj}")
                    for k in range(8):
                        nc.tensor.matmul(ps[:], wxt[k][:, ts(m, 128)],
                                         vT_c[k][:, ts(jj, 512)],
                                         start=(k == 0), stop=(k == 7))
                    nc.scalar.activation(out=xy[m][:, ts(jj, 512)], in_=ps[:],
                                         func=AF.Relu)

            def emit_rope(h, jj, xy, qrh, tg):
                sl = ts(jj, 512)
                for i in range(2):
                    t1 = sml.tile([128, 512], CDT, tag="ropet", bufs=2,
                                  name=f"t1_{tg}h{h}j{jj}i{i}")
                    nc.gpsimd.tensor_mul(t1[:], xy[i][:, sl], cosv[i][:, sl])
                    nc.vector.tensor_mul(qrh[i][:], xy[2 + i][:, sl],
                                         sinv[i][:, sl])
                    nc.vector.tensor_sub(qrh[i][:], t1[:], qrh[i][:])
                    t3 = sml.tile([128, 512], CDT, tag="ropet", bufs=2,
                                  name=f"t3_{tg}h{h}j{jj}i{i}")
                    nc.vector.tensor_mul(t3[:], xy[i][:, sl], sinv[i][:, sl])
                    nc.gpsimd.tensor_mul(qrh[2 + i][:], xy[2 + i][:, sl],
                                         cosv[i][:, sl])
                    nc.vector.tensor_add(qrh[2 + i][:], t3[:], qrh[2 + i][:])

            def emit_attn(h, j, qrA, qrB, afh, tg):
                """scores(j) -> unnormalized a(j) into afh[:, (j%2)*256] +
                rs(j); LN(a) scale applied after the y relu (rs > 0)."""
                nsb = 2 * j + 2
                rhs_q = qrA if j < 2 else qrB
                rj = ts(j % 2, 256)

                def qsl(kq, i):
                    if i < 4:
                        return qrA[kq][:, ts(i, 128)]
                    return qrB[kq][:, ts(i - 4, 128)]

                sc = []
                for i in range(nsb):
                    s = sml.tile([128, 256], CDT, tag=f"sc{i}", bufs=2,
                                 name=f"sc{i}_{tg}h{h}j{j}")
                    sc.append(s)
                    ps = PSC(f"scp_{tg}h{h}j{j}i{i}")
                    for kq in range(4):
                        nc.tensor.matmul(ps[:], qsl(kq, i), rhs_q[kq][:, rj],
                                         start=(kq == 0), stop=(kq == 3))
                    if i >= 2 * j:
                        nc.vector.tensor_mul(s[:], ps[:], masks[i - 2 * j][:])
                    else:
                        nc.scalar.activation(out=s[:], in_=ps[:], func=AF.Copy)
                sqs = []
                for d8 in range(8):
                    ap = PA(f"ap_{tg}h{h}j{j}d{d8}")
                    for i in range(nsb):
                        nc.tensor.matmul(ap[:], vn_c[i][:, ts(d8, 128)],
                                         sc[i][:],
                                         start=(i == 0), stop=(i == nsb - 1))
                    sq = sml.tile([128, 256], CDT, tag="sq", bufs=8,
                                  name=f"sq_{tg}h{h}j{j}d{d8}")
                    if d8 % 2 == 0:
                        nc.scalar.activation(out=afh[d8][:, rj], in_=ap[:],
                                             func=AF.Copy)
                        nc.gpsimd.tensor_mul(sq[:], afh[d8][:, rj],
                                             afh[d8][:, rj])
                    else:
                        nc.vector.tensor_copy(afh[d8][:, rj], ap[:])
                        nc.scalar.activation(out=sq[:], in_=ap[:],
                                             func=AF.Square)
                    sqs.append(sq)
                stp = PST(f"stp_{tg}h{h}j{j}")
                for d8 in range(8):
                    nc.tensor.matmul(stp[:], ones[:], sqs[d8][:],
                                     start=(d8 == 0), stop=(d8 == 7))
                rs = sml.tile([128, 256], FP32, tag="rs", bufs=4,
                              name=f"rs_{tg}h{h}j{j}")
                nc.scalar.activation(out=rs[:], in_=stp[:], func=AF.Sqrt,
                                     bias=epst[:], scale=1.0 / D)
                nc.vector.reciprocal(rs[:], rs[:])
                return rs

            def emit_y(h, jj, xy, afh, rss2, tg):
                """y = (relu(Wy^T @ a) * rs) * x into xy in place."""
                wyt = []
                for k in range(8):
                    w = wst.tile([128, SD], CDT, tag="wys", bufs=8,
                                 name=f"wy_{tg}h{h}j{jj}k{k}")
                    nc.sync.dma_start(w[:], wy_d[h, ts(k, 128), :])
                    wyt.append(w)
                for m in range(4):
                    ps = P5(f"yps_{tg}h{h}m{m}j{jj}")
                    for k in range(8):
                        nc.tensor.matmul(ps[:], wyt[k][:, ts(m, 128)],
                                         afh[k][:],
                                         start=(k == 0), stop=(k == 7))
                    rl = sml.tile([128, 512], CDT, tag="rl", bufs=2,
                                  name=f"rl_{tg}h{h}m{m}j{jj}")
                    nc.scalar.activation(out=rl[:], in_=ps[:], func=AF.Relu)
                    for jh in range(2):
                        nc.vector.tensor_mul(rl[:, ts(jh, 256)],
                                             rl[:, ts(jh, 256)], rss2[jh][:])
                    nc.vector.tensor_mul(xy[m][:, ts(jj, 512)], rl[:],
                                         xy[m][:, ts(jj, 512)])

            def emit_enc(ti_range, xys, ar_in, tg):
                for ti in ti_range:
                    for dh in range(2):
                        ps = PE5(f"ep_{tg}t{ti}d{dh}")
                        for kk in range(16):
                            h, m = kk // 4, kk % 4
                            nc.tensor.matmul(
                                ps[:], xys[h][m][:, ts(ti, 128)],
                                enc_t[kk][:, ts(dh, 512)],
                                start=(kk == 0), stop=(kk == 15))
                        so = stg.tile([128, 512], CDT, tag="so", bufs=2,
                                      name=f"so_{tg}t{ti}d{dh}")
                        if dh == 0:
                            nc.scalar.activation(out=so[:], in_=ps[:],
                                                 func=AF.Copy)
                        else:
                            nc.vector.tensor_copy(so[:], ps[:])
                        nc.sync.dma_start(
                            ar_in[ts(ti % 4, 128), ts(dh, 512)], so[:])

            def emit_tail(ti, ar_out, tg):
                """w chunk -> free-axis LN stats -> vnew in place."""
                wb = tlp.tile([128, D], CDT, tag="wb", bufs=3,
                              name=f"wb_{tg}t{ti}")
                nc.sync.dma_start(wb[:], ar_out[ts(ti % 4, 128), :])
                mw_r = tiny(f"mwr_{tg}t{ti}")
                nc.vector.tensor_reduce(mw_r[:], wb[:], axis=AX.X, op=ALU.add)
                tsc = tlp.tile([128, D], CDT, tag="tts", bufs=1,
                               name=f"tsc_{tg}t{ti}")
                ms_r = tiny(f"msr_{tg}t{ti}")
                nc.scalar.activation(out=tsc[:], in_=wb[:], func=AF.Square,
                                     accum_out=ms_r[:])
                tsv = tlp.tile([128, D], CDT, tag="tts", bufs=1,
                               name=f"tsv_{tg}t{ti}")
                vw_r = tiny(f"vwr_{tg}t{ti}")
                nc.gpsimd.tensor_mul(tsv[:], wb[:], vn_c[ti][:])
                nc.vector.tensor_reduce(vw_r[:], tsv[:], axis=AX.X,
                                        op=ALU.add)
                mw = tiny(f"mw_{tg}t{ti}")
                nc.scalar.activation(out=mw[:], in_=mw_r[:], func=AF.Copy,
                                     scale=1.0 / D)
                m2 = tiny(f"m2_{tg}t{ti}")
                nc.scalar.activation(out=m2[:], in_=mw_r[:], func=AF.Square,
                                     scale=1.0 / D)
                varw = tiny(f"varw_{tg}t{ti}")
                nc.scalar.activation(out=varw[:], in_=ms_r[:], func=AF.Copy,
                                     scale=1.0 / D)
                nc.vector.tensor_sub(varw[:], varw[:], m2[:])
                rsw = tiny(f"rsw_{tg}t{ti}")
                nc.scalar.activation(out=rsw[:], in_=varw[:], func=AF.Sqrt,
                                     bias=epst[:], scale=1.0)
                nc.vector.reciprocal(rsw[:], rsw[:])
                r2 = tiny(f"r2_{tg}t{ti}")
                nc.vector.tensor_mul(r2[:], rsw[:], rsw[:])
                g = tiny(f"g_{tg}t{ti}")
                nc.scalar.activation(out=g[:], in_=r2[:], func=AF.Copy,
                                     scale=-EPS, bias=2.0)
                p = tiny(f"p_{tg}t{ti}")
                nc.scalar.activation(out=p[:], in_=vw_r[:], func=AF.Copy,
                                     scale=2.0 / D)
                nc.vector.tensor_mul(p[:], p[:], rsw[:])
                msq = tiny(f"msq_{tg}t{ti}")
                nc.vector.tensor_add(msq[:], p[:], g[:])
                rss = tiny(f"rss_{tg}t{ti}")
                nc.scalar.activation(out=rss[:], in_=msq[:], func=AF.Sqrt,
                                     bias=epst[:], scale=1.0)
                nc.vector.reciprocal(rss[:], rss[:])
                bsc = tiny(f"bsc_{tg}t{ti}")
                nc.vector.tensor_mul(bsc[:], rsw[:], rss[:])
                cng = tiny(f"cng_{tg}t{ti}")
                nc.vector.tensor_mul(cng[:], mw[:], bsc[:])
                nc.scalar.activation(out=cng[:], in_=cng[:], func=AF.Copy,
                                     scale=-1.0)
                t1 = tlp.tile([128, D], CDT, tag="t1", bufs=2,
                              name=f"t1_{tg}t{ti}")
                nc.vector.tensor_scalar(t1[:], wb[:], bsc[:], cng[:],
                                        ALU.mult, ALU.add)
                t2 = tlp.tile([128, D], CDT, tag="t2", bufs=2,
                              name=f"t2_{tg}t{ti}")
                nc.scalar.activation(out=t2[:], in_=vn_c[ti][:], func=AF.Copy,
                                     scale=rss[:])
                nc.gpsimd.tensor_add(vn_c[ti][:], t1[:], t2[:])
                for a in range(8):
                    nc.sync.dma_start_transpose(
                        vT_c[a][:, ts(ti, 128)], vn_c[ti][:, ts(a, 128)])

            st = {}

            def QA(h, tg):
                return [act.tile([128, 512], CDT, tag=f"qA{i}", bufs=4,
                                 name=f"qA{i}_{tg}h{h}") for i in range(4)]

            def QB(h, tg):
                return [act.tile([128, 512], CDT, tag=f"qB{i}", bufs=2,
                                 name=f"qB{i}_{tg}h{h}") for i in range(4)]

            def pre_emit_x(tg):
                xys, qAs = [], []
                for h in range(4):
                    xy = [act.tile([128, T], CDT, tag=f"xy{h}_{m}", bufs=1,
                                   name=f"xy{h}_{m}_{tg}") for m in range(4)]
                    xys.append(xy)
                    emit_x_mm(h, 0, xy, tg)
                    qa = QA(h, tg)
                    emit_rope(h, 0, xy, qa, tg)
                    qAs.append(qa)
                st["xys"] = xys
                st["qAs"] = qAs

            def cc(ar_in, ar_out):
                nc.gpsimd.collective_compute(
                    "AllReduce", ALU.add,
                    replica_groups=[[0, 1, 2, 3], [4, 5, 6, 7]],
                    ins=[ar_in.opt()], outs=[ar_out.opt()])

            load_v("init")
            pre_emit_x("r0pre")
            pending_tail = None

            for rep in range(repeat):
                for layer in range(nlayers):
                    tg = f"r{rep}l{layer}"
                    xys, qAs = st["xys"], st["qAs"]
                    ADT = CDT
                    ar_in = [dram.tile([T2, D], ADT, tag=f"ar_in{q}",
                                       name=f"ari{q}_{tg}") for q in range(2)]
                    ar_out = [dram.tile([T2, D], ADT, tag=f"ar_out{q}",
                                        name=f"aro{q}_{tg}") for q in range(2)]
                    w0 = ar_out[0] if collective else ar_in[0]
                    w1 = ar_out[1] if collective else ar_in[1]

                    # ---- half 1: attn j0/j1, y jj0, enc t0..3, CC1 ----
                    for h in range(4):
                        afh = [act.tile([128, 512], CDT, tag=f"alA{k}",
                                        bufs=1, name=f"alA{k}_{tg}h{h}")
                               for k in range(8)]
                        rs0 = emit_attn(h, 0, qAs[h], None, afh, tg)
                        rs1 = emit_attn(h, 1, qAs[h], None, afh, tg)
                        emit_y(h, 0, xys[h], afh, [rs0, rs1], tg)
                        if h == 3 and pending_tail is not None:
                            par, ptg = pending_tail
                            for ti in range(4, 8):
                                emit_tail(ti, par, ptg)
                            pending_tail = None
                    if pending_tail is not None:
                        par, ptg = pending_tail
                        for ti in range(4, 8):
                            emit_tail(ti, par, ptg)
                        pending_tail = None
                    emit_enc(range(0, 4), xys, ar_in[0], tg)
                    if collective:
                        cc(ar_in[0], ar_out[0])

                    # ---- half 2: x jj1, attn j2/j3, y jj1 ----
                    qB_prev = None
                    for h in range(4):
                        if h == 0:
                            qb = QB(0, tg)
                            emit_x_mm(0, 1, xys[0], tg)
                            emit_rope(0, 1, xys[0], qb, tg)
                        else:
                            qb = qB_prev
                        if h < 3:
                            qb_next = QB(h + 1, tg)
                            emit_x_mm(h + 1, 1, xys[h + 1], tg)
                            emit_rope(h + 1, 1, xys[h + 1], qb_next, tg)
                            qB_prev = qb_next
                        afh = [act.tile([128, 512], CDT, tag=f"alB{k}",
                                        bufs=1, name=f"alB{k}_{tg}h{h}")
                               for k in range(8)]
                        rs2 = emit_attn(h, 2, qAs[h], qb, afh, tg)
                        rs3 = emit_attn(h, 3, qAs[h], qb, afh, tg)
                        emit_y(h, 1, xys[h], afh, [rs2, rs3], tg)

                    # ---- tail half1 (waits CC1; runs during enc t4..7) ----
                    for ti in range(0, 4):
                        emit_tail(ti, w0, tg)
                    emit_enc(range(4, 8), xys, ar_in[1], tg)
                    if collective:
                        cc(ar_in[1], ar_out[1])

                    last_layer = (layer == nlayers - 1)
                    last = last_layer and (rep == repeat - 1)
                    if not last_layer:
                        pre_emit_x(f"{tg}pre")
                        pending_tail = (w1, tg)
                    elif not last:
                        for ti in range(4, 8):
                            emit_tail(ti, w1, tg)
                        load_v(f"r{rep+1}")
                        pre_emit_x(f"r{rep+1}pre")
                    else:
                        for ti in range(4, 8):
                            emit_tail(ti, w1, tg)

            if do_readout:
                for nn_ in range(NVC):
                    rot = []
                    for k in range(8):
                        rtile = wst.tile([128, 512], CDT, tag="wys", bufs=8,
                                         name=f"ro_n{nn_}k{k}")
                        nc.sync.dma_start(
                            rtile[:, 0:VCH], ro_d[ts(k, 128), ts(nn_, VCH)])
                        rot.append(rtile)
                    for m in range(8):
                        ps = P5(f"rps_n{nn_}m{m}")
                        for k in range(8):
                            nc.tensor.matmul(ps[:, 0:VCH],
                                             vT_c[k][:, ts(m, 128)],
                                             rot[k][:, 0:VCH],
                                             start=(k == 0), stop=(k == 7))
                        ot = stg.tile([128, VCH], FP32, tag="ot", bufs=2,
                                      name=f"ot_n{nn_}m{m}")
                        if m % 2 == 0:
                            nc.vector.tensor_copy(ot[:], ps[:, 0:VCH])
                        else:
                            nc.scalar.activation(out=ot[:], in_=ps[:, 0:VCH],
                                                 func=AF.Copy)
                        nc.sync.dma_start(
                            out_d[ts(m, 128), ts(nn_, VCH)], ot[:])
    nc.compile()
    return nc


def host_prep(inputs):
    idx = np.asarray(inputs["idx"])
    wte = np.asarray(inputs["wte"], np.float32)
    enc = np.asarray(inputs["encoder"], np.float32)
    dx = np.asarray(inputs["decoder_x"], np.float32)
    dy = np.asarray(inputs["decoder_y"], np.float32)
    ro = np.asarray(inputs["readout"], np.float32)
    bf = ml_dtypes.bfloat16

    perm = np.concatenate([np.arange(0, SD, 2), np.arange(1, SD, 2)])
    Wx = np.ascontiguousarray(dx[:, :, perm])
    Wy = np.ascontiguousarray(dy[:, :, perm])
    encp = np.ascontiguousarray(enc.reshape(H, SD, D)[:, perm, :])

    g = wte[idx]
    m = g.mean(-1, keepdims=True)
    var = ((g - m) ** 2).mean(-1, keepdims=True)
    v0 = (g - m) / np.sqrt(var + EPS)

    inv_freq = 1.0 / (10000.0 ** (np.arange(0, SD, 2, dtype=np.float32) / SD))
    freqs = np.arange(T, dtype=np.float32)[None, :] * inv_freq[:, None]
    cosT = np.cos(freqs).astype(np.float32)
    sinT = np.sin(freqs).astype(np.float32)

    ss, tt = np.mgrid[0:128, 0:256]
    msk = np.stack([(tt > ss), (tt > ss + 128)]).astype(np.float32)

    in_maps = []
    for c in range(NCORES):
        b, hs = c // 4, c % 4
        hsl = slice(4 * hs, 4 * hs + 4)
        v0T = np.ascontiguousarray(v0[b].T)
        in_maps.append({
            "v0t_c": v0T.astype(bf),
            "v0n_c": np.ascontiguousarray(v0[b]).astype(bf),
            "wx": Wx[hsl].astype(bf),
            "wy": Wy[hsl].astype(bf),
            "enc": np.ascontiguousarray(encp[hsl].reshape(NHC * SD, D)).astype(bf),
            "ro": np.ascontiguousarray(ro[:, VSH * hs: VSH * (hs + 1)]).astype(bf),
            "cos": cosT.astype(bf),
            "sin": sinT.astype(bf),
            "msk": msk.astype(bf),
        })
    return in_maps


def make_runner(nc, n_cores=NCORES):
    import jax
    from jax.sharding import Mesh, PartitionSpec
    from jax.experimental.shard_map import shard_map

    bass2jax.install_neuronx_cc_hook()
    partition_name = nc.partition_id_tensor.name if nc.partition_id_tensor else None
    in_names, out_names, out_avals, zero_shapes = [], [], [], []
    for alloc in nc.m.functions[0].allocations:
        if not isinstance(alloc, mybir.MemoryLocationSet):
            continue
        name = alloc.memorylocations[0].name
        if alloc.kind == "ExternalInput":
            if name != partition_name:
                in_names.append(name)
        elif alloc.kind == "ExternalOutput":
            shape = tuple(alloc.tensor_shape)
            dtype = mybir.dt.np(alloc.dtype)
            out_names.append(name)
            out_avals.append(jax.core.ShapedArray(shape, dtype))
            zero_shapes.append((shape, dtype))
    n_params, n_outs = len(in_names), len(out_avals)
    all_in = list(in_names) + list(out_names)
    if partition_name is not None:
        all_in.append(partition_name)

    def _body(*args):
        operands = list(args)
        if partition_name is not None:
            operands.append(bass2jax.partition_id_tensor())
        return tuple(bass2jax._bass_exec_p.bind(
            *operands, out_avals=tuple(out_avals), in_names=tuple(all_in),
            out_names=tuple(out_names), lowering_input_output_aliases=(),
            sim_require_finite=True, sim_require_nnan=True, nc=nc))

    devices = jax.devices()[:n_cores]
    mesh = Mesh(np.asarray(devices), ("core",))
    f = jax.jit(
        shard_map(_body, mesh=mesh,
                  in_specs=(PartitionSpec("core"),) * (n_params + n_outs),
                  out_specs=(PartitionSpec("core"),) * n_outs, check_rep=False),
        keep_unused=True)

    def prep(in_maps):
        concat = [np.concatenate([np.asarray(in_maps[c][k])
                                  for c in range(n_cores)], axis=0)
                  for k in in_names]
        zeros = [np.zeros((n_cores * s[0], *s[1:]), dt) for (s, dt) in zero_shapes]
        return [jax.device_put(x) for x in concat + zeros]

    def run(dev_args):
        outs = f(*dev_args)
        jax.block_until_ready(outs)
        return outs

    def split(outs):
        return [{name: np.asarray(outs[i]).reshape(n_cores, *out_avals[i].shape)[c]
                 for i, name in enumerate(out_names)} for c in range(n_cores)]

    return run, prep, split


def kernel(**inputs) -> np.ndarray:
    if "prog" not in _CACHE:
        nc = build_program()
        _CACHE["prog"] = nc
        _CACHE["runner"] = make_runner(nc)
    run, prep, split = _CACHE["runner"]
    in_maps = host_prep(inputs)
    args = prep(in_maps)
    res = split(run(args))
    out = np.zeros((B, T, VOCAB), np.float32)
    for c in range(NCORES):
        b, hs = c // 4, c % 4
        out[b, :, VSH * hs: VSH * (hs + 1)] = res[c]["logits"]
    return out
